# revision 2
# baseline (speedup 1.0000x reference)
"""Trainium2 Bass kernel for the AttentionBlock problem.

Full inputs -> full output. Internally sharded across 8 NeuronCores:
core c computes output rows [1024*c, 1024*(c+1)) (sequence-parallel over
queries); every core receives the full x so no on-device collectives are
needed.

Per-core algorithm (N=8192 keys, Nq=1024 queries, d=64):
  Qs^T = (R/8)^T x_q^T, K^T = E^T x^T          (fp32 PE matmuls)
  hi/lo fp16 split of Qs^T and K^T:
    hi  = fp16(P)  via ACT copy from PSUM
    lo  = fp16(P - hi) via a second PE accumulation (re-project, then
          accumulate -I @ hi) + ACT copy -- no DVE subtracts, keeping the
          DVE free for the pass-1 max reductions.
    kt_ss (hi + ones row) is duplicated from kt_hl's hi rows by an
    SBUF->SBUF DMA (DMA engines are otherwise idle).
  pass 1 (per 128-query row tile): m_q = max_k (Q_hi K_hi^T) over fp16-hi
    scores (PE matmuls into [128,1024] PSUM + DVE reduce_max).
  pass 2 (per 512-query chunk, transposed):
    S^T_shifted = [K_hi;K_lo]^T [Q_hi;Q_hi] + [K_hi;ones]^T [Q_lo;-m]
    P^T = exp(S^T_shifted) on ACT (fp16), out_aug^T += x_aug^T P^T
  out = out_aug^T[0:64] / out_aug^T[64]  (PE transpose + reciprocal + mul)

The ones column of x_aug makes row 64 of out_aug^T the softmax
denominator; the -m row applies the max shift inside the matmul (softmax
is shift-invariant, so fp16-hi max error only moves the shift).

A block of dummy warm-up matmuls runs during the initial DMA so the PE
HAM clock gate is released (2.4 GHz) before real work arrives.
"""

import numpy as np
from contextlib import ExitStack

import concourse.bass as bass
import concourse.tile as tile
from concourse import bacc, mybir

N = 8192
D = 64
DA = D + 1
NCORES = 8
NQ = N // NCORES          # 1024 queries per core
NKB = N // 128            # 64 key blocks
NSC = N // 512            # 16 key chunks of 512
NKQ = 8                   # kt tile count (1024 keys each)
KW = N // NKQ             # 1024
QC = 512                  # query chunk (pass-2 moving dim)
NQC = NQ // QC            # 2
NRT = QC // 128           # row-tiles per chunk (4)
NG = N // 1024            # pass-1 reduce groups per row-tile (8)
NWARM = 40                # PE warm-up matmuls

F32 = mybir.dt.float32
F16 = mybir.dt.float16


def build():
    nc = bacc.Bacc("TRN2", target_bir_lowering=False, debug=False, num_devices=1)

    xT_ap = nc.dram_tensor("xT", [D, N], F32, kind="ExternalInput").ap()
    xqT_ap = nc.dram_tensor("xqT", [D, NQ], F32, kind="ExternalInput").ap()
    rp_ap = nc.dram_tensor("Rp", [D, D], F32, kind="ExternalInput").ap()
    e_ap = nc.dram_tensor("E", [D, D], F32, kind="ExternalInput").ap()
    id_ap = nc.dram_tensor("ident", [128, 128], F32, kind="ExternalInput").ap()
    idn16_ap = nc.dram_tensor("identn16", [D, D], F16, kind="ExternalInput").ap()
    ones16_ap = nc.dram_tensor("ones16", [1, N], F16, kind="ExternalInput").ap()
    DP = 72                   # x_aug block stride, 16-byte aligned in fp16
    xaug_ap = nc.dram_tensor("xaug", [N, DP], F16, kind="ExternalInput").ap()
    out_ap = nc.dram_tensor("out", [NQ, D], F32, kind="ExternalOutput").ap()

    with tile.TileContext(nc) as tc, ExitStack() as ctx:
        const = ctx.enter_context(tc.tile_pool(name="const", bufs=1))
        big = ctx.enter_context(tc.tile_pool(name="big", bufs=1))
        work = ctx.enter_context(tc.tile_pool(name="work", bufs=3))
        # PSUM budget (8 banks): ps1 [128,1024] x2 = 4, mm512 [128,512] x2 = 2,
        # po [65,512] x2 = 2.
        pp1 = ctx.enter_context(tc.tile_pool(name="pp1", bufs=2, space="PSUM"))
        pp = ctx.enter_context(tc.tile_pool(name="pp", bufs=2, space="PSUM"))
        pacc = ctx.enter_context(tc.tile_pool(name="pacc", bufs=2, space="PSUM"))

        # ---------------- input loads ----------------
        # small projection weights first so the first matmuls start early
        rp_sb = const.tile([D, D], F32)
        nc.sync.dma_start(rp_sb[:], rp_ap[:])
        e_sb = const.tile([D, D], F32)
        nc.sync.dma_start(e_sb[:], e_ap[:])
        ident = const.tile([128, 128], F32)
        nc.sync.dma_start(ident[:], id_ap[:])
        idn16 = const.tile([D, D], F16)
        nc.sync.dma_start(idn16[:], idn16_ap[:])
        xqt_sb = big.tile([D, NQ], F32)
        nc.sync.dma_start(xqt_sb[:], xqT_ap[:])
        xt_sb = big.tile([D, N], F32)
        for s in range(8):
            w = N // 8
            nc.sync.dma_start(xt_sb[:, s * w:(s + 1) * w], xT_ap[:, s * w:(s + 1) * w])

        # ---------------- PE warm-up ----------------
        # ~40 back-to-back matmuls keep the PE busy through the DMA head so
        # the HAM clock gate releases (1.2 -> 2.4 GHz) before real work.
        wps = pp.tile([128, 512], F32, tag="mm512", name="warm")
        for _ in range(NWARM):
            nc.tensor.matmul(wps[0:D, 0:D], ident[0:D, 0:D], ident[0:D, 0:D],
                             start=True, stop=True)

        # x with ones column for the PV matmul, layout [128, (block, d_pad)]
        xaug_r = big.tile([128, NKB * DP], F16)
        nc.sync.dma_start(
            xaug_r[:].rearrange("p (t d) -> p t d", d=DP),
            xaug_ap.rearrange("(t p) d -> p t d", p=128))
        xaug_v = xaug_r[:].rearrange("p (t d) -> p t d", d=DP)[:, :, 0:DA]

        # ---------------- projections + hi/lo split ----------------
        # Qs^T first so pass-1 lhsT is ready early.
        qst_hh = big.tile([128, NQ], F16, name="qst_hh")   # [Q_hi; Q_hi]
        qst_l = big.tile([DA, NQ], F16, name="qst_l")      # [Q_lo; -m]
        for s in range(NQ // 512):
            sl = slice(s * 512, (s + 1) * 512)
            pq_full = pp.tile([128, 512], F32, tag="mm512", name="pq")
            pq = pq_full[0:D, :]
            nc.tensor.matmul(pq[:], rp_sb[:], xqt_sb[:, sl], start=True, stop=True)
            nc.scalar.copy(qst_hh[0:D, sl], pq[:])
            pq2_full = pp.tile([128, 512], F32, tag="mm512", name="pq2")
            pq2 = pq2_full[0:D, :]
            nc.tensor.matmul(pq2[:], rp_sb[:], xqt_sb[:, sl], start=True, stop=False)
            nc.tensor.matmul(pq2[:], idn16[:], qst_hh[0:D, sl], start=False,
                             stop=True, skip_group_check=True)
            nc.scalar.copy(qst_l[0:D, sl], pq2[:])
            nc.sync.dma_start(qst_hh[D:2 * D, sl], qst_hh[0:D, sl])

        # K^T in 8 tiles of 1024 keys so pass-1 (and its DVE reductions)
        # start as soon as the first tiles are complete.
        kt_hl = [big.tile([128, KW], F16, name=f"kt_hl{q}") for q in range(NKQ)]
        kt_ss = [big.tile([DA, KW], F16, name=f"kt_ss{q}") for q in range(NKQ)]
        for q in range(NKQ):
            qw = slice(q * KW, (q + 1) * KW)
            nc.sync.dma_start(kt_ss[q][D:DA, :].bitcast(F32),
                              ones16_ap[:, qw].bitcast(F32))
        for s in range(NSC):
            kq, so = divmod(s, NSC // NKQ)
            sl = slice(so * 512, (so + 1) * 512)
            xsl = slice(s * 512, (s + 1) * 512)
            pk_full = pp.tile([128, 512], F32, tag="mm512", name="pk")
            pk = pk_full[0:D, :]
            nc.tensor.matmul(pk[:], e_sb[:], xt_sb[:, xsl], start=True, stop=True)
            nc.scalar.copy(kt_hl[kq][0:D, sl], pk[:])
            pk2_full = pp.tile([128, 512], F32, tag="mm512", name="pk2")
            pk2 = pk2_full[0:D, :]
            nc.tensor.matmul(pk2[:], e_sb[:], xt_sb[:, xsl], start=True, stop=False)
            nc.tensor.matmul(pk2[:], idn16[:], kt_hl[kq][0:D, sl], start=False,
                             stop=True, skip_group_check=True)
            nc.scalar.copy(kt_hl[kq][D:2 * D, sl], pk2[:])
            nc.sync.dma_start(kt_ss[kq][0:D, sl], kt_hl[kq][0:D, sl])

        # -------- pass 1 for chunk 0, then pass 2 per chunk with the next
        # chunk's pass 1 interleaved into the j-loop (the tile scheduler
        # keeps the DVE reductions flowing underneath pass 2's PE/ACT work).
        mx_tiles = {}
        mxp_tiles = {}

        def emit_pass1_group(qc, gi):
            rt, g = divmod(gi, NG)
            if g == 0:
                mxp_tiles[qc] = work.tile([128, NG], F32, tag="mxp", name="mxp")
            mxp = mxp_tiles[qc]
            q0 = qc * QC + rt * 128
            ps1 = pp1.tile([128, 1024], F32, tag="ps1", name="ps1")
            for h in range(2):
                nc.tensor.matmul(ps1[:, h * 512:(h + 1) * 512],
                                 qst_hh[0:D, q0:q0 + 128],
                                 kt_hl[g][0:D, h * 512:(h + 1) * 512],
                                 start=True, stop=True)
            nc.vector.reduce_max(mxp[:, g:g + 1], ps1[:],
                                 axis=mybir.AxisListType.X)
            if g == NG - 1:
                if qc not in mx_tiles:
                    mx_tiles[qc] = work.tile([128, NRT + 32], F32,
                                             tag="mx_all", name="mx_all")
                    nc.vector.memset(mx_tiles[qc][:], 0.0)
                nc.vector.reduce_max(mx_tiles[qc][:, rt:rt + 1], mxp[:],
                                     axis=mybir.AxisListType.X, negate=True)

        def emit_max_writeback(qc):
            # PSUM/SBUF reads must start at an aligned partition, so bring
            # each row-tile's -max to partition 0 with its own 32-wide
            # (non-degenerate) PE transpose of the zero-padded max tile,
            # then copy row 0 into qst_l row 64.
            for rt in range(NRT):
                pm_full = pp.tile([128, 512], F32, tag="mm512", name="pm")
                ps_m = pm_full[0:32, 0:128]
                nc.tensor.transpose(ps_m[:], mx_tiles[qc][:, rt:rt + 32],
                                    ident[:])
                sl = slice(qc * QC + rt * 128, qc * QC + (rt + 1) * 128)
                nc.vector.tensor_copy(qst_l[D:DA, sl], ps_m[0:1, :])

        for gi in range(NRT * NG):
            emit_pass1_group(0, gi)
        emit_max_writeback(0)

        def make_normalize(qc, po):
            def norm():
                # normalize: out[q, :] = po[0:64, q] / po[64, q]
                ot = work.tile([DA, QC], F32, tag="ot")
                nc.vector.tensor_copy(ot[:], po[:])
                for h in range(QC // 128):
                    ptr_full = pp.tile([128, 512], F32, tag="mm512", name="ptr")
                    ps_t = ptr_full[:, 0:DA]
                    nc.tensor.transpose(ps_t[:], ot[:, h * 128:(h + 1) * 128],
                                        ident[0:DA, 0:DA])
                    recip = work.tile([128, 1], F32, tag="recip")
                    nc.vector.reciprocal(recip[:], ps_t[:, D:DA])
                    o_sb = work.tile([128, D], F32, tag="o_sb")
                    nc.vector.tensor_scalar_mul(o_sb[:], ps_t[:, 0:D], recip[:])
                    r0 = qc * QC + h * 128
                    nc.sync.dma_start(out_ap[r0:r0 + 128, :], o_sb[:])
            return norm

        prev_norm = None
        for qc in range(NQC):
            # pass 2, software-pipelined at emission so the PE order is
            # S_0, S_1, PV_0, S_2, PV_1, ... (PE never waits on an exp)
            po = pacc.tile([DA, QC], F32, tag="po")

            def emit_st(j):
                ps = pp.tile([128, QC], F32, tag="mm512", name="ps_st")
                kq, jo = divmod(j, NKB // NKQ)
                blk = slice(jo * 128, (jo + 1) * 128)
                qsl = slice(qc * QC, (qc + 1) * QC)
                nc.tensor.matmul(ps[:], kt_hl[kq][:, blk], qst_hh[:, qsl],
                                 start=True, stop=False)
                nc.tensor.matmul(ps[:], kt_ss[kq][:, blk], qst_l[:, qsl],
                                 start=False, stop=True)
                return ps

            ps_cur = emit_st(0)
            for j in range(NKB):
                pt = work.tile([128, QC], F16, tag="pt")
                nc.scalar.activation(pt[:], ps_cur[:],
                                     mybir.ActivationFunctionType.Exp)
                if j + 1 < NKB:
                    ps_cur = emit_st(j + 1)
                nc.tensor.matmul(po[:], xaug_v[:, j, :], pt[:],
                                 start=(j == 0), stop=(j == NKB - 1))
                if j == 3 and prev_norm is not None:
                    prev_norm()
                    prev_norm = None
                if qc + 1 < NQC and j < NRT * NG:
                    emit_pass1_group(qc + 1, j)
            if qc + 1 < NQC:
                emit_max_writeback(qc + 1)
            prev_norm = make_normalize(qc, po)
        prev_norm()

    nc.compile()
    return nc


_CACHE = {}


def _get_nc():
    if "nc" not in _CACHE:
        _CACHE["nc"] = build()
    return _CACHE["nc"]


def kernel(x, rotation_params, entangle_params, _trace=False, _nc=None):
    from concourse.bass_utils import run_bass_kernel_spmd

    x = np.ascontiguousarray(x, dtype=np.float32)
    rp = np.ascontiguousarray(rotation_params, dtype=np.float32) / 8.0
    e = np.ascontiguousarray(entangle_params, dtype=np.float32)
    xT = np.ascontiguousarray(x.T)

    nc = _nc if _nc is not None else _get_nc()
    ones16 = np.ones((1, N), dtype=np.float16)
    xaug16 = np.zeros((N, 72), dtype=np.float16)
    xaug16[:, :D] = x.astype(np.float16)
    xaug16[:, D] = 1.0

    in_maps = []
    for c in range(NCORES):
        in_maps.append({
            "xT": xT,
            "xqT": np.ascontiguousarray(xT[:, c * NQ:(c + 1) * NQ]),
            "Rp": rp,
            "E": e,
            "ident": np.eye(128, dtype=np.float32),
            "identn16": (-np.eye(D)).astype(np.float16),
            "ones16": ones16,
            "xaug": xaug16,
        })
    res = run_bass_kernel_spmd(nc, in_maps, core_ids=list(range(NCORES)),
                               trace=_trace)
    out = np.concatenate([res.results[c]["out"] for c in range(NCORES)], axis=0)
    if _trace:
        return out, res
    return out


# revision 3
# speedup vs baseline: 1.1454x; 1.1454x over previous
"""Trainium2 Bass kernel for the AttentionBlock problem.

Full inputs -> full output. Internally sharded across 8 NeuronCores:
core c computes output rows [1024*c, 1024*(c+1)) (sequence-parallel over
queries); every core receives the full x so no on-device collectives are
needed.

Per-core algorithm (N=8192 keys, Nq=1024 queries, d=64):
  Qs^T = (R/8)^T x_q^T, K^T = E^T x^T          (fp32 PE matmuls)
  hi/lo fp16 split of Qs^T and K^T:
    hi   = fp16(P) via ACT copy from PSUM; a second ACT copy parks the
           fp32 projection in SBUF scratch so the lo subtracts (DVE) can
           be deferred until after chunk-0's max reductions -- the DVE
           reduce stream is the phase-1 critical path and must not carry
           the subtracts.
    dups = kt_ss/qst_hh second halves come from SBUF->SBUF DMA (idle
           DMA engines), not ACT.
  pass 1 (per 128-query row tile): m_q = max_k (Q_hi K_hi^T) over fp16-hi
    scores (PE matmuls into [128,1024] PSUM + DVE reduce_max); reads only
    DMA/ACT-produced tiles so it never waits on the DVE subtracts.
  pass 2 (per 512-query chunk, transposed):
    S^T_shifted = [K_hi;K_lo]^T [Q_hi;Q_hi] + [K_hi;ones]^T [Q_lo;-m]
    P^T = exp(S^T_shifted) on ACT (fp16), out_aug^T += x_aug^T P^T
  out = out_aug^T[0:64] / out_aug^T[64]  (PE transpose + reciprocal + mul)

The ones column of x_aug makes row 64 of out_aug^T the softmax
denominator; the -m row applies the max shift inside the matmul (softmax
is shift-invariant, so fp16-hi max error only moves the shift).
"""

import numpy as np
from contextlib import ExitStack

import concourse.bass as bass
import concourse.tile as tile
from concourse import bacc, mybir

N = 8192
D = 64
DA = D + 1
NCORES = 8
NQ = N // NCORES          # 1024 queries per core
NKB = N // 128            # 64 key blocks
NSC = N // 512            # 16 key chunks of 512
NKQ = 8                   # kt tile count (1024 keys each)
KW = N // NKQ             # 1024
QC = 512                  # query chunk (pass-2 moving dim)
NQC = NQ // QC            # 2
NRT = QC // 128           # row-tiles per chunk (4)
NG = N // 1024            # pass-1 reduce groups per row-tile (8)

F32 = mybir.dt.float32
F16 = mybir.dt.float16


def build():
    nc = bacc.Bacc("TRN2", target_bir_lowering=False, debug=False, num_devices=1)

    xT_ap = nc.dram_tensor("xT", [D, N], F32, kind="ExternalInput").ap()
    xqT_ap = nc.dram_tensor("xqT", [D, NQ], F32, kind="ExternalInput").ap()
    rp_ap = nc.dram_tensor("Rp", [D, D], F32, kind="ExternalInput").ap()
    e_ap = nc.dram_tensor("E", [D, D], F32, kind="ExternalInput").ap()
    id_ap = nc.dram_tensor("ident", [128, 128], F32, kind="ExternalInput").ap()
    ones16_ap = nc.dram_tensor("ones16", [1, N], F16, kind="ExternalInput").ap()
    DP = 72                   # x_aug block stride, 16-byte aligned in fp16
    xaug_ap = nc.dram_tensor("xaug", [N, DP], F16, kind="ExternalInput").ap()
    out_ap = nc.dram_tensor("out", [NQ, D], F32, kind="ExternalOutput").ap()

    with tile.TileContext(nc) as tc, ExitStack() as ctx:
        const = ctx.enter_context(tc.tile_pool(name="const", bufs=1))
        big = ctx.enter_context(tc.tile_pool(name="big", bufs=1))
        work = ctx.enter_context(tc.tile_pool(name="work", bufs=3))
        # PSUM budget (8 banks): ps1 [128,1024] x2 = 4, mm512 [128,512] x2 = 2,
        # po [65,512] x2 = 2.
        pp1 = ctx.enter_context(tc.tile_pool(name="pp1", bufs=2, space="PSUM"))
        pp = ctx.enter_context(tc.tile_pool(name="pp", bufs=2, space="PSUM"))
        pacc = ctx.enter_context(tc.tile_pool(name="pacc", bufs=2, space="PSUM"))

        # ---------------- input loads ----------------
        # small projection weights first so the first matmuls start early
        rp_sb = const.tile([D, D], F32)
        nc.sync.dma_start(rp_sb[:], rp_ap[:])
        e_sb = const.tile([D, D], F32)
        nc.sync.dma_start(e_sb[:], e_ap[:])
        ident = const.tile([128, 128], F32)
        nc.sync.dma_start(ident[:], id_ap[:])
        xqt_sb = big.tile([D, NQ], F32)
        nc.sync.dma_start(xqt_sb[:], xqT_ap[:])
        xt_sb = big.tile([D, N], F32)
        for s in range(8):
            w = N // 8
            nc.sync.dma_start(xt_sb[:, s * w:(s + 1) * w], xT_ap[:, s * w:(s + 1) * w])

        # x with ones column for the PV matmul, layout [128, (block, d_pad)]
        xaug_r = big.tile([128, NKB * DP], F16)
        nc.sync.dma_start(
            xaug_r[:].rearrange("p (t d) -> p t d", d=DP),
            xaug_ap.rearrange("(t p) d -> p t d", p=128))
        xaug_v = xaug_r[:].rearrange("p (t d) -> p t d", d=DP)[:, :, 0:DA]

        # ---------------- projections + hi/lo split ----------------
        # Qs^T first so pass-1 lhsT is ready early.
        qst_hh = big.tile([128, NQ], F16, name="qst_hh")   # [Q_hi; Q_hi]
        qst_l = big.tile([DA, NQ], F16, name="qst_l")      # [Q_lo; -m]
        q32 = big.tile([D, NQ], F32, name="q32")           # fp32 Q scratch
        for s in range(NQ // 512):
            sl = slice(s * 512, (s + 1) * 512)
            pq_full = pp.tile([128, 512], F32, tag="mm512", name="pq")
            pq = pq_full[0:D, :]
            nc.tensor.matmul(pq[:], rp_sb[:], xqt_sb[:, sl], start=True, stop=True)
            nc.scalar.copy(qst_hh[0:D, sl], pq[:])
            nc.scalar.copy(q32[:, sl], pq[:])
            nc.sync.dma_start(qst_hh[D:2 * D, sl], qst_hh[0:D, sl])

        # K^T in 8 tiles of 1024 keys so pass-1 (and its DVE reductions)
        # start as soon as the first tiles are complete.  kt_ss (hi+ones)
        # is produced purely by ACT+DMA, so pass 1 never waits on the DVE.
        kt_hl = [big.tile([128, KW], F16, name=f"kt_hl{q}") for q in range(NKQ)]
        kt_ss = [big.tile([DA, KW], F16, name=f"kt_ss{q}") for q in range(NKQ)]
        k32 = big.tile([D, N], F32, name="k32")            # fp32 K scratch
        for q in range(NKQ):
            qw = slice(q * KW, (q + 1) * KW)
            nc.sync.dma_start(kt_ss[q][D:DA, :].bitcast(F32),
                              ones16_ap[:, qw].bitcast(F32))
        for s in range(NSC):
            kq, so = divmod(s, NSC // NKQ)
            sl = slice(so * 512, (so + 1) * 512)
            xsl = slice(s * 512, (s + 1) * 512)
            pk_full = pp.tile([128, 512], F32, tag="mm512", name="pk")
            pk = pk_full[0:D, :]
            nc.tensor.matmul(pk[:], e_sb[:], xt_sb[:, xsl], start=True, stop=True)
            nc.scalar.copy(kt_ss[kq][0:D, sl], pk[:])
            nc.scalar.copy(k32[:, xsl], pk[:])
            nc.sync.dma_start(kt_hl[kq][0:D, sl], kt_ss[kq][0:D, sl])

        # lo-residual subtracts (DVE, from SBUF scratch).  Emitted after the
        # projections; the tile scheduler can slide them behind the pass-1
        # reduce stream since nothing in pass 1 reads kt_hl/qst_l lo rows.
        def emit_qsubs():
            for s in range(NQ // 512):
                sl = slice(s * 512, (s + 1) * 512)
                nc.vector.tensor_tensor(
                    out=qst_l[0:D, sl], in0=q32[:, sl], in1=qst_hh[0:D, sl],
                    op=mybir.AluOpType.subtract)

        def emit_ksub(s):
            kq, so = divmod(s, NSC // NKQ)
            sl = slice(so * 512, (so + 1) * 512)
            xsl = slice(s * 512, (s + 1) * 512)
            nc.vector.tensor_tensor(
                out=kt_hl[kq][D:2 * D, sl], in0=k32[:, xsl],
                in1=kt_ss[kq][0:D, sl], op=mybir.AluOpType.subtract)

        # -------- pass 1 for chunk 0, then pass 2 per chunk with the next
        # chunk's pass 1 interleaved into the j-loop.
        mx_tiles = {}
        mxp_tiles = {}

        def emit_pass1_group(qc, gi):
            rt, g = divmod(gi, NG)
            if g == 0:
                mxp_tiles[qc] = work.tile([128, NG], F32, tag="mxp", name="mxp")
            mxp = mxp_tiles[qc]
            q0 = qc * QC + rt * 128
            ps1 = pp1.tile([128, 1024], F32, tag="ps1", name="ps1")
            for h in range(2):
                nc.tensor.matmul(ps1[:, h * 512:(h + 1) * 512],
                                 qst_hh[0:D, q0:q0 + 128],
                                 kt_ss[g][0:D, h * 512:(h + 1) * 512],
                                 start=True, stop=True)
            nc.vector.reduce_max(mxp[:, g:g + 1], ps1[:],
                                 axis=mybir.AxisListType.X)
            if g == NG - 1:
                if qc not in mx_tiles:
                    mx_tiles[qc] = work.tile([128, NRT + 32], F32,
                                             tag="mx_all", name="mx_all")
                    nc.vector.memset(mx_tiles[qc][:], 0.0)
                nc.vector.reduce_max(mx_tiles[qc][:, rt:rt + 1], mxp[:],
                                     axis=mybir.AxisListType.X, negate=True)

        def emit_max_writeback(qc):
            # PSUM/SBUF reads must start at an aligned partition, so bring
            # each row-tile's -max to partition 0 with its own 32-wide
            # (non-degenerate) PE transpose of the zero-padded max tile,
            # then copy row 0 into qst_l row 64.
            for rt in range(NRT):
                pm_full = pp.tile([128, 512], F32, tag="mm512", name="pm")
                ps_m = pm_full[0:32, 0:128]
                nc.tensor.transpose(ps_m[:], mx_tiles[qc][:, rt:rt + 32],
                                    ident[:])
                sl = slice(qc * QC + rt * 128, qc * QC + (rt + 1) * 128)
                nc.vector.tensor_copy(qst_l[D:DA, sl], ps_m[0:1, :])

        for gi in range(NRT * NG):
            emit_pass1_group(0, gi)
        emit_max_writeback(0)
        emit_qsubs()
        for s in range(NSC):
            emit_ksub(s)

        def make_normalize(qc, po):
            def norm():
                # normalize: out[q, :] = po[0:64, q] / po[64, q]
                ot = work.tile([DA, QC], F32, tag="ot")
                nc.vector.tensor_copy(ot[:], po[:])
                for h in range(QC // 128):
                    ptr_full = pp.tile([128, 512], F32, tag="mm512", name="ptr")
                    ps_t = ptr_full[:, 0:DA]
                    nc.tensor.transpose(ps_t[:], ot[:, h * 128:(h + 1) * 128],
                                        ident[0:DA, 0:DA])
                    recip = work.tile([128, 1], F32, tag="recip")
                    nc.vector.reciprocal(recip[:], ps_t[:, D:DA])
                    o_sb = work.tile([128, D], F32, tag="o_sb")
                    nc.vector.tensor_scalar_mul(o_sb[:], ps_t[:, 0:D], recip[:])
                    r0 = qc * QC + h * 128
                    nc.sync.dma_start(out_ap[r0:r0 + 128, :], o_sb[:])
            return norm

        prev_norm = None
        for qc in range(NQC):
            # pass 2, software-pipelined at emission so the PE order is
            # S_0, S_1, PV_0, S_2, PV_1, ... (PE never waits on an exp)
            po = pacc.tile([DA, QC], F32, tag="po")

            def emit_st(j):
                ps = pp.tile([128, QC], F32, tag="mm512", name="ps_st")
                kq, jo = divmod(j, NKB // NKQ)
                blk = slice(jo * 128, (jo + 1) * 128)
                qsl = slice(qc * QC, (qc + 1) * QC)
                nc.tensor.matmul(ps[:], kt_hl[kq][:, blk], qst_hh[:, qsl],
                                 start=True, stop=False)
                nc.tensor.matmul(ps[:], kt_ss[kq][:, blk], qst_l[:, qsl],
                                 start=False, stop=True)
                return ps

            ps_cur = emit_st(0)
            for j in range(NKB):
                pt = work.tile([128, QC], F16, tag="pt")
                nc.scalar.activation(pt[:], ps_cur[:],
                                     mybir.ActivationFunctionType.Exp)
                if j + 1 < NKB:
                    ps_cur = emit_st(j + 1)
                nc.tensor.matmul(po[:], xaug_v[:, j, :], pt[:],
                                 start=(j == 0), stop=(j == NKB - 1))
                if j == 3 and prev_norm is not None:
                    prev_norm()
                    prev_norm = None
                if qc + 1 < NQC and j < NRT * NG:
                    emit_pass1_group(qc + 1, j)
            if qc + 1 < NQC:
                emit_max_writeback(qc + 1)
            prev_norm = make_normalize(qc, po)
        prev_norm()

    nc.compile()
    return nc


_CACHE = {}


def _get_nc():
    if "nc" not in _CACHE:
        _CACHE["nc"] = build()
    return _CACHE["nc"]


def kernel(x, rotation_params, entangle_params, _trace=False, _nc=None):
    from concourse.bass_utils import run_bass_kernel_spmd

    x = np.ascontiguousarray(x, dtype=np.float32)
    rp = np.ascontiguousarray(rotation_params, dtype=np.float32) / 8.0
    e = np.ascontiguousarray(entangle_params, dtype=np.float32)
    xT = np.ascontiguousarray(x.T)

    nc = _nc if _nc is not None else _get_nc()
    ones16 = np.ones((1, N), dtype=np.float16)
    xaug16 = np.zeros((N, 72), dtype=np.float16)
    xaug16[:, :D] = x.astype(np.float16)
    xaug16[:, D] = 1.0

    in_maps = []
    for c in range(NCORES):
        in_maps.append({
            "xT": xT,
            "xqT": np.ascontiguousarray(xT[:, c * NQ:(c + 1) * NQ]),
            "Rp": rp,
            "E": e,
            "ident": np.eye(128, dtype=np.float32),
            "ones16": ones16,
            "xaug": xaug16,
        })
    res = run_bass_kernel_spmd(nc, in_maps, core_ids=list(range(NCORES)),
                               trace=_trace)
    out = np.concatenate([res.results[c]["out"] for c in range(NCORES)], axis=0)
    if _trace:
        return out, res
    return out


# revision 5
# speedup vs baseline: 1.4720x; 1.2852x over previous
"""Trainium2 Bass kernel for the AttentionBlock problem.

Full inputs -> full output. Internally sharded across 8 NeuronCores:
core c computes output rows [1024*c, 1024*(c+1)) (sequence-parallel over
queries); every core receives the full x so no on-device collectives are
needed.

Per-core algorithm (N=8192 keys, Nq=1024 queries, d=64):
  Qs^T = (R/8)^T x_q^T, K^T = E^T x^T          (fp32 PE matmuls)
  hi/lo fp16 split of Qs^T and K^T:
    hi   = fp16(P) via ACT copy from PSUM; a second ACT copy parks the
           fp32 projection in SBUF scratch so the lo subtracts (DVE) can
           be deferred until after chunk-0's max reductions -- the DVE
           reduce stream is the phase-1 critical path and must not carry
           the subtracts.
    dups = kt_ss/qst_hh second halves come from SBUF->SBUF DMA (idle
           DMA engines), not ACT.
  pass 1 (per 128-query row tile): m_q = max_k (Q_hi K_hi^T) over fp16-hi
    scores (PE matmuls into [128,1024] PSUM + DVE reduce_max); reads only
    DMA/ACT-produced tiles so it never waits on the DVE subtracts.
  pass 2 (per 512-query chunk, transposed):
    S^T_shifted = [K_hi;K_lo]^T [Q_hi;Q_hi] + [K_hi;ones]^T [Q_lo;-m]
    P^T = exp(S^T_shifted) on ACT (fp16), out_aug^T += x_aug^T P^T
  out = out_aug^T[0:64] / out_aug^T[64]  (PE transpose + reciprocal + mul)

The ones column of x_aug makes row 64 of out_aug^T the softmax
denominator; the -m row applies the max shift inside the matmul (softmax
is shift-invariant, so fp16-hi max error only moves the shift).
"""

import numpy as np
from contextlib import ExitStack

import concourse.bass as bass
import concourse.tile as tile
from concourse import bacc, mybir

N = 8192
D = 64
DA = D + 1
NCORES = 8
NQ = N // NCORES          # 1024 queries per core
NKB = N // 128            # 64 key blocks
NSC = N // 512            # 16 key chunks of 512
NKQ = 8                   # kt tile count (1024 keys each)
KW = N // NKQ             # 1024
QC = 512                  # query chunk (pass-2 moving dim)
NQC = NQ // QC            # 2
NRT = QC // 128           # row-tiles per chunk (4)
NG = N // 1024            # pass-1 reduce groups per row-tile (8)

F32 = mybir.dt.float32
F16 = mybir.dt.float16


def build():
    nc = bacc.Bacc("TRN2", target_bir_lowering=False, debug=False, num_devices=1)

    xT_ap = nc.dram_tensor("xT", [D, N], F32, kind="ExternalInput").ap()
    xqT_ap = nc.dram_tensor("xqT", [D, NQ], F32, kind="ExternalInput").ap()
    rp_ap = nc.dram_tensor("Rp", [D, D], F32, kind="ExternalInput").ap()
    e_ap = nc.dram_tensor("E", [D, D], F32, kind="ExternalInput").ap()
    id_ap = nc.dram_tensor("ident", [128, 128], F32, kind="ExternalInput").ap()
    ones16_ap = nc.dram_tensor("ones16", [1, N], F16, kind="ExternalInput").ap()
    DP = 72                   # x_aug block stride, 16-byte aligned in fp16
    xaug_ap = nc.dram_tensor("xaug", [N, DP], F16, kind="ExternalInput").ap()
    out_ap = nc.dram_tensor("out", [NQ, D], F32, kind="ExternalOutput").ap()

    with tile.TileContext(nc) as tc, ExitStack() as ctx:
        const = ctx.enter_context(tc.tile_pool(name="const", bufs=1))
        big = ctx.enter_context(tc.tile_pool(name="big", bufs=1))
        work = ctx.enter_context(tc.tile_pool(name="work", bufs=3))
        # PSUM budget (8 banks): ps1 [128,1024] x2 = 4, mm512 [128,512] x3 = 3,
        # po [65,512] x1 = 1.  Three mm512 bufs let pass 2 run a 3-deep score
        # pipeline (S(j+2) emitted before PV(j)) so the exp latency never
        # stalls the PE at the warm clock.
        pp1 = ctx.enter_context(tc.tile_pool(name="pp1", bufs=2, space="PSUM"))
        pp = ctx.enter_context(tc.tile_pool(name="pp", bufs=3, space="PSUM"))
        pacc = ctx.enter_context(tc.tile_pool(name="pacc", bufs=1, space="PSUM"))

        # ---------------- input loads ----------------
        # small projection weights first so the first matmuls start early
        rp_sb = const.tile([D, D], F32)
        nc.sync.dma_start(rp_sb[:], rp_ap[:])
        e_sb = const.tile([D, D], F32)
        nc.sync.dma_start(e_sb[:], e_ap[:])
        ident = const.tile([128, 128], F32)
        nc.sync.dma_start(ident[:], id_ap[:])
        xqt_sb = big.tile([D, NQ], F32)
        nc.sync.dma_start(xqt_sb[:], xqT_ap[:])
        xt_sb = big.tile([D, N], F32)
        for s in range(8):
            w = N // 8
            nc.sync.dma_start(xt_sb[:, s * w:(s + 1) * w], xT_ap[:, s * w:(s + 1) * w])

        # x with ones column for the PV matmul, layout [128, (block, d_pad)]
        xaug_r = big.tile([128, NKB * DP], F16)
        nc.sync.dma_start(
            xaug_r[:].rearrange("p (t d) -> p t d", d=DP),
            xaug_ap.rearrange("(t p) d -> p t d", p=128))
        xaug_v = xaug_r[:].rearrange("p (t d) -> p t d", d=DP)[:, :, 0:DA]

        # ---------------- projections + hi/lo split ----------------
        # Qs^T first so pass-1 lhsT is ready early.
        qst_hh = big.tile([128, NQ], F16, name="qst_hh")   # [Q_hi; Q_hi]
        qst_l = big.tile([DA, NQ], F16, name="qst_l")      # [Q_lo; -m]
        q32 = big.tile([D, NQ], F32, name="q32")           # fp32 Q scratch
        for s in range(NQ // 512):
            sl = slice(s * 512, (s + 1) * 512)
            pq_full = pp.tile([128, 512], F32, tag="mm512", name="pq")
            pq = pq_full[0:D, :]
            nc.tensor.matmul(pq[:], rp_sb[:], xqt_sb[:, sl], start=True, stop=True)
            nc.scalar.copy(qst_hh[0:D, sl], pq[:])
            nc.scalar.copy(q32[:, sl], pq[:])
            nc.sync.dma_start(qst_hh[D:2 * D, sl], qst_hh[0:D, sl])

        # K^T in 8 tiles of 1024 keys so pass-1 (and its DVE reductions)
        # start as soon as the first tiles are complete.  kt_ss (hi+ones)
        # is produced purely by ACT+DMA, so pass 1 never waits on the DVE.
        kt_hl = [big.tile([128, KW], F16, name=f"kt_hl{q}") for q in range(NKQ)]
        kt_ss = [big.tile([DA, KW], F16, name=f"kt_ss{q}") for q in range(NKQ)]
        k32 = big.tile([D, N], F32, name="k32")            # fp32 K scratch
        for q in range(NKQ):
            qw = slice(q * KW, (q + 1) * KW)
            nc.sync.dma_start(kt_ss[q][D:DA, :].bitcast(F32),
                              ones16_ap[:, qw].bitcast(F32))
        for s in range(NSC):
            kq, so = divmod(s, NSC // NKQ)
            sl = slice(so * 512, (so + 1) * 512)
            xsl = slice(s * 512, (s + 1) * 512)
            pk_full = pp.tile([128, 512], F32, tag="mm512", name="pk")
            pk = pk_full[0:D, :]
            nc.tensor.matmul(pk[:], e_sb[:], xt_sb[:, xsl], start=True, stop=True)
            nc.scalar.copy(kt_ss[kq][0:D, sl], pk[:])
            nc.scalar.copy(k32[:, xsl], pk[:])
            nc.sync.dma_start(kt_hl[kq][0:D, sl], kt_ss[kq][0:D, sl])

        # lo-residual subtracts (DVE, from SBUF scratch).  Emitted after the
        # projections; the tile scheduler can slide them behind the pass-1
        # reduce stream since nothing in pass 1 reads kt_hl/qst_l lo rows.
        def emit_qsubs():
            for s in range(NQ // 512):
                sl = slice(s * 512, (s + 1) * 512)
                nc.vector.tensor_tensor(
                    out=qst_l[0:D, sl], in0=q32[:, sl], in1=qst_hh[0:D, sl],
                    op=mybir.AluOpType.subtract)

        def emit_ksub(s):
            kq, so = divmod(s, NSC // NKQ)
            sl = slice(so * 512, (so + 1) * 512)
            xsl = slice(s * 512, (s + 1) * 512)
            nc.vector.tensor_tensor(
                out=kt_hl[kq][D:2 * D, sl], in0=k32[:, xsl],
                in1=kt_ss[kq][0:D, sl], op=mybir.AluOpType.subtract)

        # -------- pass 1 for chunk 0, then pass 2 per chunk with the next
        # chunk's pass 1 interleaved into the j-loop.
        mx_tiles = {}
        mxp_tiles = {}

        def emit_pass1_group(qc, gi):
            rt, g = divmod(gi, NG)
            if g == 0:
                mxp_tiles[qc] = work.tile([128, NG], F32, tag="mxp", name="mxp")
            mxp = mxp_tiles[qc]
            q0 = qc * QC + rt * 128
            ps1 = pp1.tile([128, 1024], F32, tag="ps1", name="ps1")
            for h in range(2):
                nc.tensor.matmul(ps1[:, h * 512:(h + 1) * 512],
                                 qst_hh[0:D, q0:q0 + 128],
                                 kt_ss[g][0:D, h * 512:(h + 1) * 512],
                                 start=True, stop=True)
            nc.vector.reduce_max(mxp[:, g:g + 1], ps1[:],
                                 axis=mybir.AxisListType.X)
            if g == NG - 1:
                if qc not in mx_tiles:
                    mx_tiles[qc] = work.tile([128, NRT + 32], F32,
                                             tag="mx_all", name="mx_all")
                    nc.vector.memset(mx_tiles[qc][:], 0.0)
                nc.vector.reduce_max(mx_tiles[qc][:, rt:rt + 1], mxp[:],
                                     axis=mybir.AxisListType.X, negate=True)

        def emit_max_writeback(qc):
            # PSUM/SBUF reads must start at an aligned partition, so bring
            # each row-tile's -max to partition 0 with its own 32-wide
            # (non-degenerate) PE transpose of the zero-padded max tile,
            # then copy row 0 into qst_l row 64.
            for rt in range(NRT):
                pm_full = pp.tile([128, 512], F32, tag="mm512", name="pm")
                ps_m = pm_full[0:32, 0:128]
                nc.tensor.transpose(ps_m[:], mx_tiles[qc][:, rt:rt + 32],
                                    ident[:])
                sl = slice(qc * QC + rt * 128, qc * QC + (rt + 1) * 128)
                nc.vector.tensor_copy(qst_l[D:DA, sl], ps_m[0:1, :])

        for gi in range(NRT * NG):
            emit_pass1_group(0, gi)
        emit_max_writeback(0)
        emit_qsubs()
        for s in range(NSC):
            emit_ksub(s)

        def make_normalize(qc, po):
            def norm():
                # normalize: out[q, :] = po[0:64, q] / po[64, q]
                ot = work.tile([DA, QC], F32, tag="ot")
                nc.vector.tensor_copy(ot[:], po[:])
                for h in range(QC // 128):
                    ptr_full = pp.tile([128, 512], F32, tag="mm512", name="ptr")
                    ps_t = ptr_full[:, 0:DA]
                    nc.tensor.transpose(ps_t[:], ot[:, h * 128:(h + 1) * 128],
                                        ident[0:DA, 0:DA])
                    recip = work.tile([128, 1], F32, tag="recip")
                    nc.vector.reciprocal(recip[:], ps_t[:, D:DA])
                    o_sb = work.tile([128, D], F32, tag="o_sb")
                    nc.vector.tensor_scalar_mul(o_sb[:], ps_t[:, 0:D], recip[:])
                    r0 = qc * QC + h * 128
                    nc.sync.dma_start(out_ap[r0:r0 + 128, :], o_sb[:])
            return norm

        # Interleave the next chunk's pass-1 groups over the first ~45 blocks
        # of this chunk's j-loop: dense enough that the writeback lands well
        # before the chunk boundary, sparse enough that the DVE reduce
        # stream (1.2us each) keeps up with the PE at the warm clock.
        P1_SPREAD = 45
        p1_sched = {}
        for gi in range(NRT * NG):
            p1_sched.setdefault(gi * P1_SPREAD // (NRT * NG), []).append(gi)

        prev_norm = None
        for qc in range(NQC):
            # pass 2 with a 3-deep score pipeline: PE order per iteration is
            # S(j+2), PV(j), so exp(j) has ~2 blocks of PE time to complete
            # and the PE never waits on the ACT.
            if prev_norm is not None:
                prev_norm()
                prev_norm = None
            po = pacc.tile([DA, QC], F32, tag="po")

            def emit_st(j):
                ps = pp.tile([128, QC], F32, tag="mm512", name="ps_st")
                kq, jo = divmod(j, NKB // NKQ)
                blk = slice(jo * 128, (jo + 1) * 128)
                qsl = slice(qc * QC, (qc + 1) * QC)
                nc.tensor.matmul(ps[:], kt_hl[kq][:, blk], qst_hh[:, qsl],
                                 start=True, stop=False)
                nc.tensor.matmul(ps[:], kt_ss[kq][:, blk], qst_l[:, qsl],
                                 start=False, stop=True)
                return ps

            ps_q = [emit_st(0), emit_st(1)]
            for j in range(NKB):
                pt = work.tile([128, QC], F16, tag="pt")
                nc.scalar.activation(pt[:], ps_q.pop(0)[:],
                                     mybir.ActivationFunctionType.Exp)
                if j + 2 < NKB:
                    ps_q.append(emit_st(j + 2))
                nc.tensor.matmul(po[:], xaug_v[:, j, :], pt[:],
                                 start=(j == 0), stop=(j == NKB - 1))
                if qc + 1 < NQC:
                    for gi in p1_sched.get(j, []):
                        emit_pass1_group(qc + 1, gi)
            if qc + 1 < NQC:
                emit_max_writeback(qc + 1)
            prev_norm = make_normalize(qc, po)
        prev_norm()

    nc.compile()
    return nc


_CACHE = {}


def _get_nc():
    if "nc" not in _CACHE:
        _CACHE["nc"] = build()
    return _CACHE["nc"]


def kernel(x, rotation_params, entangle_params, _trace=False, _nc=None):
    from concourse.bass_utils import run_bass_kernel_spmd

    x = np.ascontiguousarray(x, dtype=np.float32)
    rp = np.ascontiguousarray(rotation_params, dtype=np.float32) / 8.0
    e = np.ascontiguousarray(entangle_params, dtype=np.float32)
    xT = np.ascontiguousarray(x.T)

    nc = _nc if _nc is not None else _get_nc()
    ones16 = np.ones((1, N), dtype=np.float16)
    xaug16 = np.zeros((N, 72), dtype=np.float16)
    xaug16[:, :D] = x.astype(np.float16)
    xaug16[:, D] = 1.0

    in_maps = []
    for c in range(NCORES):
        in_maps.append({
            "xT": xT,
            "xqT": np.ascontiguousarray(xT[:, c * NQ:(c + 1) * NQ]),
            "Rp": rp,
            "E": e,
            "ident": np.eye(128, dtype=np.float32),
            "ones16": ones16,
            "xaug": xaug16,
        })
    res = run_bass_kernel_spmd(nc, in_maps, core_ids=list(range(NCORES)),
                               trace=_trace)
    out = np.concatenate([res.results[c]["out"] for c in range(NCORES)], axis=0)
    if _trace:
        return out, res
    return out


# revision 11
# speedup vs baseline: 1.4918x; 1.0135x over previous
"""Trainium2 Bass kernel for the AttentionBlock problem.

Full inputs -> full output. Internally sharded across 8 NeuronCores:
core c computes output rows [1024*c, 1024*(c+1)) (sequence-parallel over
queries); every core receives the full x so no on-device collectives are
needed.

Per-core algorithm (N=8192 keys, Nq=1024 queries, d=64):
  Qs^T = (R/8)^T x_q^T, K^T = E^T x^T          (fp32 PE matmuls)
  hi/lo fp16 split of Qs^T and K^T:
    hi   = fp16(P) via ACT copy from PSUM; a second ACT copy parks the
           fp32 projection in SBUF scratch so the lo subtracts (DVE) can
           be deferred until after chunk-0's max reductions -- the DVE
           reduce stream is the phase-1 critical path and must not carry
           the subtracts.
    dups = kt_ss/qst_hh second halves come from SBUF->SBUF DMA (idle
           DMA engines), not ACT.
  pass 1 (per 128-query row tile): m_q = max_k (Q_hi K_hi^T) over fp16-hi
    scores (PE matmuls into [128,1024] PSUM + DVE reduce_max); reads only
    DMA/ACT-produced tiles so it never waits on the DVE subtracts.
  pass 2 (per 512-query chunk, transposed):
    S^T_shifted = [K_hi;K_lo]^T [Q_hi;Q_hi] + [K_hi;ones]^T [Q_lo;-m]
    P^T = exp(S^T_shifted) on ACT (fp16), out_aug^T += x_aug^T P^T
  out = out_aug^T[0:64] / out_aug^T[64]  (PE transpose + reciprocal + mul)

The ones column of x_aug makes row 64 of out_aug^T the softmax
denominator; the -m row applies the max shift inside the matmul (softmax
is shift-invariant, so fp16-hi max error only moves the shift).
"""

import numpy as np
from contextlib import ExitStack

import concourse.bass as bass
import concourse.tile as tile
from concourse import bacc, mybir

N = 8192
D = 64
DA = D + 1
NCORES = 8
NQ = N // NCORES          # 1024 queries per core
NKB = N // 128            # 64 key blocks
NSC = N // 512            # 16 key chunks of 512
NKQ = 8                   # kt tile count (1024 keys each)
KW = N // NKQ             # 1024
QC = 512                  # query chunk (pass-2 moving dim)
NQC = NQ // QC            # 2
NRT = QC // 128           # row-tiles per chunk (4)
NG = N // 1024            # pass-1 reduce groups per row-tile (8)

F32 = mybir.dt.float32
F16 = mybir.dt.float16


def build():
    nc = bacc.Bacc("TRN2", target_bir_lowering=False, debug=False, num_devices=1)

    xT_ap = nc.dram_tensor("xT", [D, N], F32, kind="ExternalInput").ap()
    xqT_ap = nc.dram_tensor("xqT", [D, NQ], F32, kind="ExternalInput").ap()
    rp_ap = nc.dram_tensor("Rp", [D, D], F32, kind="ExternalInput").ap()
    e_ap = nc.dram_tensor("E", [D, D], F32, kind="ExternalInput").ap()
    id_ap = nc.dram_tensor("ident", [128, 128], F32, kind="ExternalInput").ap()
    ones16_ap = nc.dram_tensor("ones16", [1, N], F16, kind="ExternalInput").ap()
    DP = 72                   # x_aug block stride, 16-byte aligned in fp16
    xaug_ap = nc.dram_tensor("xaug", [N, DP], F16, kind="ExternalInput").ap()
    out_ap = nc.dram_tensor("out", [NQ, D], F32, kind="ExternalOutput").ap()

    with tile.TileContext(nc) as tc, ExitStack() as ctx:
        const = ctx.enter_context(tc.tile_pool(name="const", bufs=1))
        big = ctx.enter_context(tc.tile_pool(name="big", bufs=1))
        work = ctx.enter_context(tc.tile_pool(name="work", bufs=3))
        # PSUM budget (8 banks): ps1 [128,1024] x2 = 4, mm512 [128,512] x3 = 3,
        # po [65,512] x1 = 1.  Three mm512 bufs let pass 2 run a 3-deep score
        # pipeline (S(j+2) emitted before PV(j)) so the exp latency never
        # stalls the PE at the warm clock.
        pp1 = ctx.enter_context(tc.tile_pool(name="pp1", bufs=2, space="PSUM"))
        pp = ctx.enter_context(tc.tile_pool(name="pp", bufs=3, space="PSUM"))
        pacc = ctx.enter_context(tc.tile_pool(name="pacc", bufs=1, space="PSUM"))

        # ---------------- input loads ----------------
        # DMA descriptor issue costs ~0.3-1us per dma_start on the issuing
        # engine's queue, so the loads are spread across engines: the
        # critical projection chain (rp, e, xqT, xT) on Sync in need-order;
        # later-needed tensors (ident, ones16, xaug) on Vector/GpSimd.
        # (issue in first-use order: per-queue DMA bandwidth is ~17 GB/s and
        # packets drain in issue order, so early bytes gate the first MMs)
        rp_sb = const.tile([D, D], F32)
        nc.sync.dma_start(rp_sb[:], rp_ap[:])
        xqt_sb = big.tile([D, NQ], F32)
        xt_sb = big.tile([D, N], F32)
        w = NQ // 2
        nc.sync.dma_start(xqt_sb[:, 0:w], xqT_ap[:, 0:w])
        e_sb = const.tile([D, D], F32)
        nc.sync.dma_start(e_sb[:], e_ap[:])
        wt = N // 8
        nc.sync.dma_start(xt_sb[:, 0:wt], xT_ap[:, 0:wt])
        nc.sync.dma_start(xqt_sb[:, w:NQ], xqT_ap[:, w:NQ])
        for s in range(1, 8):
            nc.sync.dma_start(xt_sb[:, s * wt:(s + 1) * wt],
                              xT_ap[:, s * wt:(s + 1) * wt])
        ident = const.tile([128, 128], F32)
        nc.gpsimd.dma_start(ident[:], id_ap[:])
        # K tiles declared here so the ones rows issue on GpSimd before the
        # descriptor-heavy xaug load.
        kt_hl = [big.tile([128, KW], F16, name=f"kt_hl{q}") for q in range(NKQ)]
        kt_ss = [big.tile([DA, KW], F16, name=f"kt_ss{q}") for q in range(NKQ)]
        for q in range(NKQ):
            qw = slice(q * KW, (q + 1) * KW)
            nc.gpsimd.dma_start(kt_ss[q][D:DA, :].bitcast(F32),
                                ones16_ap[:, qw].bitcast(F32))

        # x with ones column for the PV matmul, layout [128, (block, d_pad)]
        xaug_r = big.tile([128, NKB * DP], F16)
        nc.gpsimd.dma_start(
            xaug_r[:].rearrange("p (t d) -> p t d", d=DP),
            xaug_ap.rearrange("(t p) d -> p t d", p=128))
        xaug_v = xaug_r[:].rearrange("p (t d) -> p t d", d=DP)[:, :, 0:DA]

        # ---------------- projections + hi/lo split ----------------
        # Qs^T first so pass-1 lhsT is ready early.
        qst_hh = big.tile([128, NQ], F16, name="qst_hh")   # [Q_hi; Q_hi]
        qst_l = big.tile([DA, NQ], F16, name="qst_l")      # [Q_lo; -m]
        q32 = big.tile([D, NQ], F32, name="q32")           # fp32 Q scratch
        for s in range(NQ // 512):
            sl = slice(s * 512, (s + 1) * 512)
            pq_full = pp.tile([128, 512], F32, tag="mm512", name="pq")
            pq = pq_full[0:D, :]
            nc.tensor.matmul(pq[:], rp_sb[:], xqt_sb[:, sl], start=True, stop=True)
            nc.scalar.copy(qst_hh[0:D, sl], pq[:])
            nc.scalar.copy(q32[:, sl], pq[:])
            nc.sync.dma_start(qst_hh[D:2 * D, sl], qst_hh[0:D, sl])

        # K^T in 8 tiles of 1024 keys so pass-1 (and its DVE reductions)
        # start as soon as the first tiles are complete.  kt_ss (hi+ones)
        # is produced purely by ACT+DMA, so pass 1 never waits on the DVE.
        k32 = big.tile([D, N], F32, name="k32")            # fp32 K scratch
        for s in range(NSC):
            kq, so = divmod(s, NSC // NKQ)
            sl = slice(so * 512, (so + 1) * 512)
            xsl = slice(s * 512, (s + 1) * 512)
            pk_full = pp.tile([128, 512], F32, tag="mm512", name="pk")
            pk = pk_full[0:D, :]
            nc.tensor.matmul(pk[:], e_sb[:], xt_sb[:, xsl], start=True, stop=True)
            nc.scalar.copy(kt_ss[kq][0:D, sl], pk[:])
            nc.scalar.copy(k32[:, xsl], pk[:])
            nc.sync.dma_start(kt_hl[kq][0:D, sl], kt_ss[kq][0:D, sl])

        # lo-residual subtracts (DVE, from SBUF scratch).  Emitted after the
        # projections; the tile scheduler can slide them behind the pass-1
        # reduce stream since nothing in pass 1 reads kt_hl/qst_l lo rows.
        def emit_qsubs():
            for s in range(NQ // 512):
                sl = slice(s * 512, (s + 1) * 512)
                nc.vector.tensor_tensor(
                    out=qst_l[0:D, sl], in0=q32[:, sl], in1=qst_hh[0:D, sl],
                    op=mybir.AluOpType.subtract)

        def emit_ksub(s):
            kq, so = divmod(s, NSC // NKQ)
            sl = slice(so * 512, (so + 1) * 512)
            xsl = slice(s * 512, (s + 1) * 512)
            nc.vector.tensor_tensor(
                out=kt_hl[kq][D:2 * D, sl], in0=k32[:, xsl],
                in1=kt_ss[kq][0:D, sl], op=mybir.AluOpType.subtract)

        # -------- pass 1 for chunk 0, then pass 2 per chunk with the next
        # chunk's pass 1 interleaved into the j-loop.
        mx_tiles = {}
        mxp_tiles = {}

        def emit_pass1_group(qc, gi):
            rt, g = divmod(gi, NG)
            if g == 0:
                mxp_tiles[qc] = work.tile([128, NG], F32, tag="mxp", name="mxp")
            mxp = mxp_tiles[qc]
            q0 = qc * QC + rt * 128
            ps1 = pp1.tile([128, 1024], F32, tag="ps1", name="ps1")
            for h in range(2):
                nc.tensor.matmul(ps1[:, h * 512:(h + 1) * 512],
                                 qst_hh[0:D, q0:q0 + 128],
                                 kt_ss[g][0:D, h * 512:(h + 1) * 512],
                                 start=True, stop=True)
            nc.vector.reduce_max(mxp[:, g:g + 1], ps1[:],
                                 axis=mybir.AxisListType.X)
            if g == NG - 1:
                if qc not in mx_tiles:
                    mx_tiles[qc] = work.tile([128, NRT + 32], F32,
                                             tag="mx_all", name="mx_all")
                    nc.vector.memset(mx_tiles[qc][:], 0.0)
                nc.vector.reduce_max(mx_tiles[qc][:, rt:rt + 1], mxp[:],
                                     axis=mybir.AxisListType.X, negate=True)

        def emit_max_writeback(qc):
            # PSUM/SBUF reads must start at an aligned partition, so bring
            # each row-tile's -max to partition 0 with its own 32-wide
            # (non-degenerate) PE transpose of the zero-padded max tile,
            # then copy row 0 into qst_l row 64.
            for rt in range(NRT):
                pm_full = pp.tile([128, 512], F32, tag="mm512", name="pm")
                ps_m = pm_full[0:32, 0:128]
                nc.tensor.transpose(ps_m[:], mx_tiles[qc][:, rt:rt + 32],
                                    ident[:])
                sl = slice(qc * QC + rt * 128, qc * QC + (rt + 1) * 128)
                nc.vector.tensor_copy(qst_l[D:DA, sl], ps_m[0:1, :])

        for gi in range(NRT * NG):
            emit_pass1_group(0, gi)
        emit_max_writeback(0)
        emit_qsubs()
        for s in range(NSC):
            emit_ksub(s)

        def make_normalize(qc, po):
            def norm():
                # normalize: out[q, :] = po[0:64, q] / po[64, q]
                ot = work.tile([DA, QC], F32, tag="ot")
                nc.vector.tensor_copy(ot[:], po[:])
                for h in range(QC // 128):
                    ptr_full = pp.tile([128, 512], F32, tag="mm512", name="ptr")
                    ps_t = ptr_full[:, 0:DA]
                    nc.tensor.transpose(ps_t[:], ot[:, h * 128:(h + 1) * 128],
                                        ident[0:DA, 0:DA])
                    recip = work.tile([128, 1], F32, tag="recip")
                    nc.vector.reciprocal(recip[:], ps_t[:, D:DA])
                    o_sb = work.tile([128, D], F32, tag="o_sb")
                    nc.vector.tensor_scalar_mul(o_sb[:], ps_t[:, 0:D], recip[:])
                    r0 = qc * QC + h * 128
                    nc.sync.dma_start(out_ap[r0:r0 + 128, :], o_sb[:])
            return norm

        # Interleave the next chunk's pass-1 groups over the first ~45 blocks
        # of this chunk's j-loop: dense enough that the writeback lands well
        # before the chunk boundary, sparse enough that the DVE reduce
        # stream (1.2us each) keeps up with the PE at the warm clock.
        P1_SPREAD = 45
        p1_sched = {}
        for gi in range(NRT * NG):
            p1_sched.setdefault(gi * P1_SPREAD // (NRT * NG), []).append(gi)

        prev_norm = None
        for qc in range(NQC):
            # pass 2 with a 3-deep score pipeline: PE order per iteration is
            # S(j+2), PV(j), so exp(j) has ~2 blocks of PE time to complete
            # and the PE never waits on the ACT.
            if prev_norm is not None:
                prev_norm()
                prev_norm = None
            po = pacc.tile([DA, QC], F32, tag="po")

            def emit_st(j):
                ps = pp.tile([128, QC], F32, tag="mm512", name="ps_st")
                kq, jo = divmod(j, NKB // NKQ)
                blk = slice(jo * 128, (jo + 1) * 128)
                qsl = slice(qc * QC, (qc + 1) * QC)
                nc.tensor.matmul(ps[:], kt_hl[kq][:, blk], qst_hh[:, qsl],
                                 start=True, stop=False)
                nc.tensor.matmul(ps[:], kt_ss[kq][:, blk], qst_l[:, qsl],
                                 start=False, stop=True)
                return ps

            ps_q = [emit_st(0), emit_st(1)]
            for j in range(NKB):
                pt = work.tile([128, QC], F16, tag="pt")
                nc.scalar.activation(pt[:], ps_q.pop(0)[:],
                                     mybir.ActivationFunctionType.Exp)
                if j + 2 < NKB:
                    ps_q.append(emit_st(j + 2))
                nc.tensor.matmul(po[:], xaug_v[:, j, :], pt[:],
                                 start=(j == 0), stop=(j == NKB - 1))
                if qc + 1 < NQC:
                    for gi in p1_sched.get(j, []):
                        emit_pass1_group(qc + 1, gi)
            if qc + 1 < NQC:
                emit_max_writeback(qc + 1)
            prev_norm = make_normalize(qc, po)
        prev_norm()

    nc.compile()
    return nc


_CACHE = {}


def _get_nc():
    if "nc" not in _CACHE:
        _CACHE["nc"] = build()
    return _CACHE["nc"]


def kernel(x, rotation_params, entangle_params, _trace=False, _nc=None):
    from concourse.bass_utils import run_bass_kernel_spmd

    x = np.ascontiguousarray(x, dtype=np.float32)
    rp = np.ascontiguousarray(rotation_params, dtype=np.float32) / 8.0
    e = np.ascontiguousarray(entangle_params, dtype=np.float32)
    xT = np.ascontiguousarray(x.T)

    nc = _nc if _nc is not None else _get_nc()
    ones16 = np.ones((1, N), dtype=np.float16)
    xaug16 = np.zeros((N, 72), dtype=np.float16)
    xaug16[:, :D] = x.astype(np.float16)
    xaug16[:, D] = 1.0

    in_maps = []
    for c in range(NCORES):
        in_maps.append({
            "xT": xT,
            "xqT": np.ascontiguousarray(xT[:, c * NQ:(c + 1) * NQ]),
            "Rp": rp,
            "E": e,
            "ident": np.eye(128, dtype=np.float32),
            "ones16": ones16,
            "xaug": xaug16,
        })
    res = run_bass_kernel_spmd(nc, in_maps, core_ids=list(range(NCORES)),
                               trace=_trace)
    out = np.concatenate([res.results[c]["out"] for c in range(NCORES)], axis=0)
    if _trace:
        return out, res
    return out
